# revision 1
# baseline (speedup 1.0000x reference)
"""Trainium2 Bass kernel for GQA attention prefill (Mistral-style, RoPE, causal).

B=1, S=2048, DIM=4096, 32 Q heads / 8 KV heads, HD=128, rope theta 1e6.

Sharding: tensor-parallel over heads across 8 cores. Core i gets Q heads
4i..4i+3 and KV head i. x is replicated (pre-transposed + bf16-cast on host).
Each core computes its 4 heads' attention and a partial output projection
(contraction over its 512 input dims of wo); the host sums the 8 partials.

Per-core dataflow (all matmuls bf16 with fp32 PSUM accumulation):
  phase A (per 128-row s block):
    xT tiles [c,s] (lhsT) x wT [c, q|k|v] (rhs) -> psum [s, 768]
    rope applied in [s, d] layout via stride-2 APs (DVE), cast bf16
    PE-transpose q/k 128x128 blocks -> resident QT/KT [d, s]; V kept [s, d]
  phase B (per 512-col q chunk t, per head h):
    scores_T [k,q] = KT_tile.T @ QT  (one matmul per 128-k tile, no accum)
    P_T = exp(scale * scores_T) on ACT (no max subtraction: |scores| < ~15),
    diagonal blocks masked by precomputed 0/1 tiles (DVE)
    attn_T [d, q] accumulated via lhsT=V tiles; denom via lhsT=ones
    (every row of the denom psum equals the column sum of P_T)
    normalize on DVE (reciprocal + multiply) -> at [d', s] bf16
  o-proj: psum [s,512e] accumulated over the 4 heads, lhsT=at slices,
    rhs=woT [d', e]; evacuate fp32 and DMA to the partial output.
"""

import numpy as np
import ml_dtypes

S = 2048
DIM = 4096
HD = 128
N_CORES = 8
QH_PER_CORE = 4  # 512 q dims per core
DQ = QH_PER_CORE * HD  # 512
SCALE = 1.0 / float(np.sqrt(HD))
SB = S // 128  # 16 s blocks
CB = DIM // 128  # 32 contraction blocks
NT = S // 512  # 4 q chunks
ET = DIM // 512  # 8 e tiles

bf16 = ml_dtypes.bfloat16

_RUNNER = None


ALL_STAGES = frozenset({"proj", "rope", "tpose", "scores", "pv", "oproj"})

# debug knobs for perf isolation (set by bench scripts)
NO_EXP = False
NO_OUTDMA = False
PT_CONST = False


def _build(reps=None, stages=ALL_STAGES):
    import concourse.bass as bass
    import concourse.mybir as mybir
    import concourse.tile as tile
    from concourse import bacc
    from concourse.masks import make_identity
    from contextlib import nullcontext

    dt = mybir.dt
    Exp = mybir.ActivationFunctionType.Exp

    nc = bacc.Bacc(
        "TRN2", target_bir_lowering=False, debug=False, num_devices=N_CORES
    )

    xt_d = nc.dram_tensor("xt", [DIM, S], dt.bfloat16, kind="ExternalInput").ap()
    wt_d = nc.dram_tensor("wt", [DIM, 768], dt.bfloat16, kind="ExternalInput").ap()
    wot_d = nc.dram_tensor("wot", [DQ, DIM], dt.bfloat16, kind="ExternalInput").ap()
    csd_d = nc.dram_tensor("csd", [128, S], dt.float32, kind="ExternalInput").ap()
    snd_d = nc.dram_tensor("snd", [128, S], dt.float32, kind="ExternalInput").ap()
    mask_d = nc.dram_tensor("mask", [512, 512], dt.bfloat16, kind="ExternalInput").ap()
    out_d = nc.dram_tensor("out", [S, DIM], dt.float32, kind="ExternalOutput").ap()

    with tile.TileContext(nc) as tc:
        with tc.For_i(0, reps, 1) if reps else nullcontext(), tc.tile_pool(
            name="const", bufs=1
        ) as cp:
            mask_sb = cp.tile([128, 4, 512], dt.bfloat16)
            nc.sync.dma_start(out=mask_sb, in_=mask_d.rearrange("(m p) n -> p m n", p=128))
            ones_sb = cp.tile([128, 128], dt.float32)
            nc.vector.memset(ones_sb, 1.0)
            ptc_sb = cp.tile([128, 512], dt.bfloat16)
            nc.vector.memset(ptc_sb, 0.5)
            ident_sb = cp.tile([128, 128], dt.bfloat16)
            make_identity(nc, ident_sb)

            qt_sb = cp.tile([128, QH_PER_CORE, S], dt.bfloat16)  # [d, h, s]
            kt_sb = cp.tile([128, S], dt.bfloat16)  # [d, s]
            v_sb = cp.tile([128, SB, HD], dt.bfloat16)  # [s128, sb, d]

            # ---------------- phase A: projections + rope (direct QT) ---------
            # Weights are the stationary operand; psum comes out as [d, s]
            # (already transposed for attention). Q/K rows are host-permuted
            # per head into [even-pairs | odd-pairs] so rope works on
            # partition halves (inputs share a base; outputs may shift).
            # V is PE-transposed back to [s, d] (16 blocks).
            with (
                tc.tile_pool(name="pa", bufs=2) as pa,
                tc.tile_pool(name="pap", bufs=3, space="PSUM") as pap,
            ):
                wt_sb = pa.tile([128, CB, 768], dt.bfloat16, bufs=1)
                nc.sync.dma_start(
                    out=wt_sb, in_=wt_d.rearrange("(cb c) n -> c cb n", c=128)
                )
                csd_sb = pa.tile([128, S], dt.float32, bufs=1)
                nc.sync.dma_start(out=csd_sb, in_=csd_d)
                snd_sb = pa.tile([128, S], dt.float32, bufs=1)
                nc.sync.dma_start(out=snd_sb, in_=snd_d)

                def rope_evac(ps, dest, s0):
                    # dest[0:64]   = a*cos - b*sin
                    # dest[64:128] = a*sin + b*cos   (a=rows 0:64, b=rows 64:128)
                    cs = csd_sb[:, s0 : s0 + 512]
                    sn = snd_sb[:, s0 : s0 + 512]
                    t1 = pa.tile([128, 512], dt.float32, tag="t1")
                    t2 = pa.tile([128, 512], dt.float32, tag="t2")
                    nc.vector.tensor_mul(t1, ps, cs)
                    nc.vector.tensor_mul(t2[0:64, :], ps[64:128, :], sn[64:128, :])
                    nc.vector.tensor_mul(t2[64:128, :], ps[0:64, :], sn[0:64, :])
                    nc.vector.tensor_sub(dest[0:64, :], t1[0:64, :], t2[0:64, :])
                    nc.vector.tensor_add(dest[64:128, :], t1[64:128, :], t2[64:128, :])

                for sc in range(4):  # s chunks of 512
                    s0 = sc * 512
                    xt_sb = pa.tile([128, CB, 512], dt.bfloat16, tag="xt")
                    nc.sync.dma_start(
                        out=xt_sb,
                        in_=xt_d.rearrange("(cb c) s -> c cb s", c=128)[
                            :, :, s0 : s0 + 512
                        ],
                    )
                    for dtile in range(6):  # 4 Q heads, K, V
                        ps = pap.tile([128, 512], dt.float32, tag="proj")
                        for cb in range(CB if "proj" in stages else 0):
                            nc.tensor.matmul(
                                ps,
                                lhsT=wt_sb[:, cb, dtile * 128 : (dtile + 1) * 128],
                                rhs=xt_sb[:, cb, :],
                                start=(cb == 0),
                                stop=(cb == CB - 1),
                            )
                        if "rope" not in stages:
                            continue
                        if dtile < QH_PER_CORE:
                            rope_evac(ps, qt_sb[:, dtile, s0 : s0 + 512], s0)
                        elif dtile == QH_PER_CORE:
                            rope_evac(ps, kt_sb[:, s0 : s0 + 512], s0)
                        else:
                            vt_st = pa.tile([128, 512], dt.bfloat16, tag="vt")
                            nc.vector.tensor_copy(vt_st, ps)
                            for b in range(4 if "tpose" in stages else 0):
                                pst = pap.tile(
                                    [128, 128], dt.bfloat16, tag="tp", bufs=2
                                )
                                nc.tensor.transpose(
                                    pst, vt_st[:, b * 128 : (b + 1) * 128], ident_sb
                                )
                                nc.vector.tensor_copy(
                                    v_sb[:, sc * 4 + b, :], pst
                                )

            # ---------------- phase B: attention + output projection ----------
            # Software-pipelined: the o-projection for chunk t-1 is emitted
            # between the per-head attention groups of chunk t, so PE has
            # dense matmul work while ACT runs the exps of the current chunk.
            with (
                tc.tile_pool(name="pb", bufs=2) as pb,
                tc.tile_pool(name="pbp", bufs=2, space="PSUM") as pbp,
            ):
                woT_sb = pb.tile([128, QH_PER_CORE, DIM], dt.bfloat16, bufs=1)
                nc.sync.dma_start(
                    out=woT_sb, in_=wot_d.rearrange("(db p) e -> p db e", p=128)
                )

                def oproj_group(t, sbl, ats):
                    if "oproj" not in stages:
                        return
                    o_sb = pb.tile([128, DIM], dt.float32, tag="osb")
                    for e in range(ET):
                        ps_out = pbp.tile([128, 512], dt.float32, tag="oproj")
                        for h in range(QH_PER_CORE):
                            nc.tensor.matmul(
                                ps_out,
                                lhsT=ats[h][:, sbl * 128 : (sbl + 1) * 128],
                                rhs=woT_sb[:, h, e * 512 : (e + 1) * 512],
                                start=(h == 0),
                                stop=(h == QH_PER_CORE - 1),
                            )
                        # split evacuations between DVE and ACT (Copy is in
                        # the exp table set, so no table reload)
                        ev_eng = nc.vector.tensor_copy if e % 2 == 0 else nc.scalar.copy
                        ev_eng(o_sb[:, e * 512 : (e + 1) * 512], ps_out)
                    if not NO_OUTDMA:
                        nc.scalar.dma_start(
                            out=out_d[(4 * t + sbl) * 128 : (4 * t + sbl + 1) * 128, :],
                            in_=o_sb,
                        )

                prev_ats = None
                for t in range(NT if ("scores" in stages) else 0):
                    nkb = 4 * (t + 1)
                    at_tiles = []
                    for h in range(QH_PER_CORE):
                        qs = qt_sb[:, h, t * 512 : (t + 1) * 512]
                        ps_o = pbp.tile([128, 512], dt.float32, tag="attnT", bufs=1)
                        dacc = pb.tile([128, 512], dt.float32, tag="dacc", bufs=2)
                        for kb in range(0, nkb, 2):
                            ps_s = pbp.tile([128, 1024], dt.float32, tag="scores")
                            for j in (0, 1):
                                nc.tensor.matmul(
                                    ps_s[:, j * 512 : (j + 1) * 512],
                                    lhsT=kt_sb[:, (kb + j) * 128 : (kb + j + 1) * 128],
                                    rhs=qs,
                                    start=True,
                                    stop=True,
                                )
                            if PT_CONST:
                                pt = ptc_sb
                            else:
                                pt = pb.tile(
                                    [128, 1024], dt.bfloat16, tag="pt", bufs=4
                                )
                                if not NO_EXP:
                                    nc.scalar.activation(pt, ps_s, Exp, scale=SCALE)
                                else:
                                    nc.gpsimd.memset(pt, 0.5)
                                for j in (0, 1):
                                    if kb + j >= 4 * t:
                                        # masks run on the otherwise-idle POOL
                                        nc.gpsimd.tensor_mul(
                                            pt[:, j * 512 : (j + 1) * 512],
                                            pt[:, j * 512 : (j + 1) * 512],
                                            mask_sb[:, kb + j - 4 * t, :],
                                        )
                            if "pv" in stages:
                                for j in (0, 1):
                                    ptj = pt[:, j * 512 : (j + 1) * 512] if not PT_CONST else ptc_sb
                                    nc.tensor.matmul(
                                        ps_o,
                                        lhsT=v_sb[:, kb + j, :],
                                        rhs=ptj,
                                        start=(kb + j == 0),
                                        stop=(kb + j == nkb - 1),
                                    )
                                    # denominator partials accumulate on DVE
                                    if kb + j == 0:
                                        nc.vector.tensor_copy(dacc, ptj)
                                    else:
                                        nc.vector.tensor_add(dacc, dacc, ptj)
                        at = pb.tile([128, 512], dt.bfloat16, tag=f"at{h}")
                        if "pv" in stages:
                            # partition-reduce + broadcast the denominator in
                            # one fp32 matmul: every output row = column sum
                            ps_d = pbp.tile([128, 512], dt.float32, tag="denom", bufs=1)
                            nc.tensor.matmul(
                                ps_d, lhsT=ones_sb, rhs=dacc, start=True, stop=True
                            )
                            recip = pb.tile([128, 512], dt.float32, tag="recip")
                            nc.vector.reciprocal(recip, ps_d)
                            nc.vector.tensor_mul(at, ps_o, recip)
                        at_tiles.append(at)
                        if prev_ats is not None:
                            oproj_group(t - 1, h, prev_ats)
                    prev_ats = at_tiles
                if prev_ats is not None:
                    for sbl in range(4):
                        oproj_group(NT - 1, sbl, prev_ats)
    nc.compile()
    return nc


def _prep_inputs(x, cos, sin, wq, wk, wv, wo):
    x = np.asarray(x, dtype=np.float32)
    cos = np.asarray(cos, dtype=np.float32)
    sin = np.asarray(sin, dtype=np.float32)
    wq = np.asarray(wq, dtype=np.float32)
    wk = np.asarray(wk, dtype=np.float32)
    wv = np.asarray(wv, dtype=np.float32)
    wo = np.asarray(wo, dtype=np.float32)

    xt = np.ascontiguousarray(x[0].T).astype(bf16)  # [DIM, S]
    # cos/sin transposed and duplicated into both partition halves [128, S]
    csd = np.ascontiguousarray(np.tile(cos.T, (2, 1)).astype(np.float32))
    snd = np.ascontiguousarray(np.tile(sin.T, (2, 1)).astype(np.float32))
    # de-interleave perm: head dim pairs (2i, 2i+1) -> rows (i, 64+i)
    perm = np.concatenate([np.arange(0, HD, 2), np.arange(1, HD, 2)])

    # causal masks for the 4 diagonal sub-blocks: mask[r, c] = (r + delta) <= c
    r = np.arange(128)[:, None]
    c = np.arange(512)[None, :]
    mask = np.concatenate(
        [((r + d) <= c).astype(bf16) for d in (0, 128, 256, 384)], axis=0
    )  # [512, 512]

    in_maps = []
    for i in range(N_CORES):
        wq_i = wq[DQ * i : DQ * (i + 1)]  # [512, DIM]
        wk_i = wk[HD * i : HD * (i + 1)]  # [128, DIM]
        wv_i = wv[HD * i : HD * (i + 1)]
        wq_p = wq_i.reshape(QH_PER_CORE, HD, DIM)[:, perm, :].reshape(DQ, DIM)
        wk_p = wk_i[perm, :]
        wt = np.concatenate([wq_p.T, wk_p.T, wv_i.T], axis=1).astype(bf16)
        wot = np.ascontiguousarray(wo[:, DQ * i : DQ * (i + 1)].T).astype(
            bf16
        )  # [512, DIM]
        in_maps.append(
            {
                "xt": xt,
                "wt": np.ascontiguousarray(wt),
                "wot": wot,
                "csd": csd,
                "snd": snd,
                "mask": np.ascontiguousarray(mask),
            }
        )
    return in_maps


def _get_runner():
    global _RUNNER
    if _RUNNER is None:
        _RUNNER = _build()
    return _RUNNER


def kernel(x, cos, sin, wq, wk, wv, wo):
    from concourse.bass_utils import run_bass_kernel_spmd

    nc = _get_runner()
    in_maps = _prep_inputs(x, cos, sin, wq, wk, wv, wo)
    res = run_bass_kernel_spmd(nc, in_maps, list(range(N_CORES)))
    out = np.zeros((S, DIM), dtype=np.float32)
    for i in range(N_CORES):
        out += res.results[i]["out"]
    return out[None].astype(np.float32)



# revision 14
# speedup vs baseline: 134.6818x; 134.6818x over previous
"""Trainium2 Bass kernel for GQA attention prefill (Mistral-style, RoPE, causal).

B=1, S=2048, DIM=4096, 32 Q heads / 8 KV heads, HD=128, rope theta 1e6.

Sharding: tensor-parallel over heads across 8 cores. Core i gets Q heads
4i..4i+3 and KV head i. x is replicated (pre-transposed + bf16-cast on host).
Each core computes its 4 heads' attention and a partial output projection
(contraction over its 512 input dims of wo); the host sums the 8 partials
(partials are written bf16 to halve the output DMA).

Per-core dataflow (all matmuls bf16 with fp32 PSUM accumulation):
  phase A (per 512-col s chunk):
    xT tiles [c,s] (lhsT) x wT [c, q|k|v] (rhs) -> psum [d, s]
    (already transposed for attention). Q/K rows host-permuted per head
    into [even-pairs | odd-pairs] so rope works on partition halves.
    rope: t1 = ps*cos, t2 = ps*sin (full-partition DVE muls), then
    dest_lo = t1_lo - t2_hi, dest_hi = t2_lo + t1_hi (two half ops).
    V is PE-transposed back to [s, d]; V-path evacuations on ACT.
  phase B (per 512-col q chunk t, per head h):
    scores_T [k,q] = KT_tile.T @ QT, restricted to the causally live
    q range for diagonal k blocks; P_T = exp(scale*scores_T) on ACT
    (no max subtraction: |scores*scale| < ~10). The diagonal 128x128
    block gets a triangular 0/1 mask (DVE bf16). PV accumulates over
    restricted ranges; denominator partials accumulate in bf16 on DVE
    and are partition-reduced+broadcast by a ones matmul; normalize =
    reciprocal + multiply -> at [d, s] bf16.
  o-proj: psum [s,512e] accumulated over the 4 heads, lhsT=at slices,
    rhs=woT [d', e]; evacuated (DVE/ACT alternating) to bf16 and DMA'd
    on the sync ring. The o-projection for chunk t-1 is emitted between
    the per-head attention groups of chunk t (PE filler during exps).

Input DMAs are split into ~1-2MB pieces spread over the sync and scalar
HWDGE rings, ordered so the first projection matmul's operands land
first; the exp activation table is preloaded during the initial wait.
"""

import numpy as np
import ml_dtypes

S = 2048
DIM = 4096
HD = 128
N_CORES = 8
QH_PER_CORE = 4  # 512 q dims per core
DQ = QH_PER_CORE * HD  # 512
SCALE = 1.0 / float(np.sqrt(HD))
SB = S // 128  # 16 s blocks
CB = DIM // 128  # 32 contraction blocks
NT = S // 512  # 4 q chunks
ET = DIM // 512  # 8 e tiles

bf16 = ml_dtypes.bfloat16

_RUNNER = None


def _build(reps=None):
    import concourse.bass as bass
    import concourse.mybir as mybir
    import concourse.tile as tile
    from concourse import bacc
    from concourse.masks import make_identity
    from contextlib import nullcontext

    dt = mybir.dt
    Exp = mybir.ActivationFunctionType.Exp

    nc = bacc.Bacc(
        "TRN2", target_bir_lowering=False, debug=False, num_devices=N_CORES
    )

    xt_d = nc.dram_tensor("xt", [DIM, S], dt.bfloat16, kind="ExternalInput").ap()
    wt_d = nc.dram_tensor("wt", [DIM, 768], dt.bfloat16, kind="ExternalInput").ap()
    wot_d = nc.dram_tensor("wot", [DQ, DIM], dt.bfloat16, kind="ExternalInput").ap()
    csd_d = nc.dram_tensor("csd", [128, S], dt.float32, kind="ExternalInput").ap()
    snd_d = nc.dram_tensor("snd", [128, S], dt.float32, kind="ExternalInput").ap()
    tri_d = nc.dram_tensor("tri", [128, 128], dt.bfloat16, kind="ExternalInput").ap()
    out_d = nc.dram_tensor("out", [S, DIM], dt.bfloat16, kind="ExternalOutput").ap()

    xt_r = xt_d.rearrange("(cb c) s -> c cb s", c=128)
    wt_r = wt_d.rearrange("(cb c) n -> c cb n", c=128)

    with tile.TileContext(nc) as tc:
        with tc.For_i(0, reps, 1) if reps else nullcontext(), tc.tile_pool(
            name="const", bufs=1
        ) as cp:
            ones_sb = cp.tile([128, 128], dt.bfloat16)
            nc.vector.memset(ones_sb, 1.0)
            # preload the exp table set during the initial DMA wait
            warm_sb = cp.tile([128, 8], dt.float32)
            nc.scalar.activation(warm_sb, ones_sb[:, 0:8], Exp)
            ident_sb = cp.tile([128, 128], dt.bfloat16)
            make_identity(nc, ident_sb)
            tri_sb = cp.tile([128, 128], dt.bfloat16)
            nc.scalar.dma_start(out=tri_sb, in_=tri_d)

            qt_sb = cp.tile([128, QH_PER_CORE, S], dt.bfloat16)  # [d, h, s]
            kt_sb = cp.tile([128, S], dt.bfloat16)  # [d, s]
            v_sb = cp.tile([128, SB, HD], dt.bfloat16)  # [s128, sb, d]
            woT_sb = cp.tile([128, QH_PER_CORE, DIM], dt.bfloat16)
            wot_r = wot_d.rearrange("(db p) e -> p db e", p=128)

            # ---------------- phase A: projections + rope (direct QT) ---------
            with (
                tc.tile_pool(name="pa", bufs=2) as pa,
                tc.tile_pool(name="pap", bufs=3, space="PSUM") as pap,
            ):
                wt_sb = pa.tile([128, CB, 768], dt.bfloat16, bufs=1)
                csd_sb = pa.tile([128, S], dt.float32, bufs=1)
                snd_sb = pa.tile([128, S], dt.float32, bufs=1)
                # startup-ordered DMA pieces across the two HWDGE rings:
                # the first projection group's operands (wt dtile 0 + xt
                # chunk 0) land first, the rest stream in behind them.
                xt0_sb = pa.tile([128, CB, 512], dt.bfloat16, name="xt0",
                                 tag="xt", bufs=2)
                nc.sync.dma_start(out=wt_sb[:, :, 0:128], in_=wt_r[:, :, 0:128])
                for i in range(4):  # xt chunk 0 in 8-cb pieces, 2 per ring
                    eng = nc.sync if i < 2 else nc.scalar
                    eng.dma_start(
                        out=xt0_sb[:, 8 * i : 8 * (i + 1), :],
                        in_=xt_r[:, 8 * i : 8 * (i + 1), 0:512],
                    )
                nc.sync.dma_start(out=wt_sb[:, :, 128:256], in_=wt_r[:, :, 128:256])
                nc.scalar.dma_start(out=wt_sb[:, :, 256:512], in_=wt_r[:, :, 256:512])
                nc.sync.dma_start(out=wt_sb[:, :, 512:768], in_=wt_r[:, :, 512:768])
                nc.scalar.dma_start(out=csd_sb, in_=csd_d)
                nc.scalar.dma_start(out=snd_sb, in_=snd_d)
                nc.sync.dma_start(out=woT_sb[:, :, 0:2048], in_=wot_r[:, :, 0:2048])
                nc.sync.dma_start(out=woT_sb[:, :, 2048:4096], in_=wot_r[:, :, 2048:4096])

                def rope_evac(ps, dest, s0):
                    # dest[0:64]   = a*cos - b*sin
                    # dest[64:128] = a*sin + b*cos   (a=rows 0:64, b=rows 64:128)
                    # DVE 2-input SBUF ops need equal base partitions on the
                    # inputs (outputs may shift), so the sin products are
                    # computed half-wise with the cross-half move on the
                    # output side.
                    cs = csd_sb[:, s0 : s0 + 512]
                    sn = snd_sb[:, s0 : s0 + 512]
                    t1 = pa.tile([128, 512], dt.float32, tag="t1")
                    t2 = pa.tile([128, 512], dt.float32, tag="t2")
                    nc.vector.tensor_mul(t1, ps, cs)
                    nc.vector.tensor_mul(t2[0:64, :], ps[64:128, :], sn[64:128, :])
                    nc.vector.tensor_mul(t2[64:128, :], ps[0:64, :], sn[0:64, :])
                    nc.vector.tensor_sub(dest[0:64, :], t1[0:64, :], t2[0:64, :])
                    nc.vector.tensor_add(dest[64:128, :], t1[64:128, :], t2[64:128, :])

                for sc in range(4):  # s chunks of 512
                    s0 = sc * 512
                    if sc == 0:
                        xt_sb = xt0_sb
                    else:
                        xt_sb = pa.tile([128, CB, 512], dt.bfloat16,
                                        name=f"xt{sc}", tag="xt", bufs=2)
                        for half, eng in ((0, nc.sync), (1, nc.scalar)):
                            eng.dma_start(
                                out=xt_sb[:, 16 * half : 16 * (half + 1), :],
                                in_=xt_r[:, 16 * half : 16 * (half + 1), s0 : s0 + 512],
                            )
                    for dtile in range(6):  # 4 Q heads, K, V
                        ps = pap.tile([128, 512], dt.float32, tag="proj")
                        # first group: visit cb blocks in DMA-arrival order
                        cbs = (
                            list(range(16, 24)) + list(range(0, 8))
                            + list(range(24, 32)) + list(range(8, 16))
                            if sc == 0 and dtile == 0
                            else range(CB)
                        )
                        for ci, cb in enumerate(cbs):
                            nc.tensor.matmul(
                                ps,
                                lhsT=wt_sb[:, cb, dtile * 128 : (dtile + 1) * 128],
                                rhs=xt_sb[:, cb, :],
                                start=(ci == 0),
                                stop=(ci == CB - 1),
                            )
                        if dtile < QH_PER_CORE:
                            rope_evac(ps, qt_sb[:, dtile, s0 : s0 + 512], s0)
                        elif dtile == QH_PER_CORE:
                            rope_evac(ps, kt_sb[:, s0 : s0 + 512], s0)
                        else:
                            vt_st = pa.tile([128, 512], dt.bfloat16, tag="vt")
                            nc.scalar.copy(vt_st, ps)
                            for b in range(4):
                                pst = pap.tile(
                                    [128, 128], dt.bfloat16, tag="tp", bufs=2
                                )
                                nc.tensor.transpose(
                                    pst, vt_st[:, b * 128 : (b + 1) * 128], ident_sb
                                )
                                nc.scalar.copy(v_sb[:, sc * 4 + b, :], pst)

            # ---------------- phase B: attention + output projection ----------
            with (
                tc.tile_pool(name="pb", bufs=2) as pb,
                tc.tile_pool(name="pbp", bufs=2, space="PSUM") as pbp,
            ):
                def oproj_group(t, sbl, ats):
                    o_sb = pb.tile([128, DIM], dt.bfloat16, tag="osb")
                    row = (4 * t + sbl) * 128
                    for e in range(ET):
                        ps_out = pbp.tile([128, 512], dt.float32, tag="oproj")
                        for h in range(QH_PER_CORE):
                            nc.tensor.matmul(
                                ps_out,
                                lhsT=ats[h][:, sbl * 128 : (sbl + 1) * 128],
                                rhs=woT_sb[:, h, e * 512 : (e + 1) * 512],
                                start=(h == 0),
                                stop=(h == QH_PER_CORE - 1),
                            )
                        # split evacuations between DVE and ACT
                        ev_eng = nc.vector.tensor_copy if e % 2 == 0 else nc.scalar.copy
                        ev_eng(o_sb[:, e * 512 : (e + 1) * 512], ps_out)
                        if e == ET // 2 - 1:  # first half out early
                            nc.sync.dma_start(
                                out=out_d[row : row + 128, 0 : DIM // 2],
                                in_=o_sb[:, 0 : DIM // 2],
                            )
                    nc.sync.dma_start(
                        out=out_d[row : row + 128, DIM // 2 : DIM],
                        in_=o_sb[:, DIM // 2 : DIM],
                    )

                prev_ats = None
                for t in range(NT):
                    nkb = 4 * (t + 1)
                    at_tiles = []
                    for h in range(QH_PER_CORE):
                        qs = qt_sb[:, h, t * 512 : (t + 1) * 512]
                        ps_o = pbp.tile([128, 512], dt.float32, tag="attnT", bufs=1)
                        dacc = pb.tile([128, 512], dt.bfloat16, tag="dacc", bufs=2)
                        for kb0 in range(0, nkb, 2):
                            diag = kb0 >= 4 * t
                            ps_s = pbp.tile([128, 1024], dt.float32, tag="scores")
                            for j in (0, 1):
                                kb = kb0 + j
                                qlo = 128 * (kb - 4 * t) if diag else 0
                                nc.tensor.matmul(
                                    ps_s[:, j * 512 + qlo : (j + 1) * 512],
                                    lhsT=kt_sb[:, kb * 128 : (kb + 1) * 128],
                                    rhs=qs[:, qlo:512],
                                    start=True,
                                    stop=True,
                                )
                            pt = pb.tile([128, 1024], dt.bfloat16, tag="pt", bufs=4)
                            if diag:
                                # per-kb exp over exactly the written range
                                for j in (0, 1):
                                    qlo = 128 * (kb0 + j - 4 * t)
                                    nc.scalar.activation(
                                        pt[:, j * 512 + qlo : (j + 1) * 512],
                                        ps_s[:, j * 512 + qlo : (j + 1) * 512],
                                        Exp,
                                        scale=SCALE,
                                    )
                            else:
                                nc.scalar.activation(pt, ps_s, Exp, scale=SCALE)
                            for j in (0, 1):
                                kb = kb0 + j
                                off = j * 512
                                if kb >= 4 * t:
                                    qlo = 128 * (kb - 4 * t)
                                    # mask the triangular 128x128 diagonal block
                                    nc.vector.tensor_mul(
                                        pt[:, off + qlo : off + qlo + 128],
                                        pt[:, off + qlo : off + qlo + 128],
                                        tri_sb,
                                    )
                                else:
                                    qlo = 0
                                nc.tensor.matmul(
                                    ps_o[:, qlo:512],
                                    lhsT=v_sb[:, kb, :],
                                    rhs=pt[:, off + qlo : off + 512],
                                    start=(kb == 0),
                                    stop=(kb == nkb - 1),
                                )
                                # denominator partials accumulate in bf16 on DVE
                                if kb == 0:
                                    nc.vector.tensor_copy(dacc, pt[:, 0:512])
                                else:
                                    nc.vector.tensor_add(
                                        dacc[:, qlo:512],
                                        dacc[:, qlo:512],
                                        pt[:, off + qlo : off + 512],
                                    )
                        # partition-reduce + broadcast the denominator in
                        # one matmul: every output row = the column sum
                        ps_d = pbp.tile([128, 512], dt.float32, tag="denom", bufs=1)
                        nc.tensor.matmul(
                            ps_d, lhsT=ones_sb, rhs=dacc, start=True, stop=True
                        )
                        recip = pb.tile([128, 512], dt.float32, tag="recip")
                        nc.vector.reciprocal(recip, ps_d)
                        at = pb.tile([128, 512], dt.bfloat16, tag=f"at{h}")
                        nc.vector.tensor_mul(at, ps_o, recip)
                        at_tiles.append(at)
                        if prev_ats is not None:
                            oproj_group(t - 1, h, prev_ats)
                    prev_ats = at_tiles
                for sbl in range(4):
                    oproj_group(NT - 1, sbl, prev_ats)
    nc.compile()
    return nc


def _prep_inputs(x, cos, sin, wq, wk, wv, wo):
    x = np.asarray(x, dtype=np.float32)
    cos = np.asarray(cos, dtype=np.float32)
    sin = np.asarray(sin, dtype=np.float32)
    wq = np.asarray(wq, dtype=np.float32)
    wk = np.asarray(wk, dtype=np.float32)
    wv = np.asarray(wv, dtype=np.float32)
    wo = np.asarray(wo, dtype=np.float32)

    xt = np.ascontiguousarray(x[0].T).astype(bf16)  # [DIM, S]
    # cos/sin transposed and duplicated into both partition halves [128, S]
    csd = np.ascontiguousarray(np.tile(cos.T, (2, 1)).astype(np.float32))
    snd = np.ascontiguousarray(np.tile(sin.T, (2, 1)).astype(np.float32))
    # de-interleave perm: head dim pairs (2i, 2i+1) -> rows (i, 64+i)
    perm = np.concatenate([np.arange(0, HD, 2), np.arange(1, HD, 2)])

    # causal triangular mask for a diagonal 128x128 block: keep k <= q
    r = np.arange(128)[:, None]
    c = np.arange(128)[None, :]
    tri = np.ascontiguousarray((r <= c).astype(bf16))

    in_maps = []
    for i in range(N_CORES):
        wq_i = wq[DQ * i : DQ * (i + 1)]  # [512, DIM]
        wk_i = wk[HD * i : HD * (i + 1)]  # [128, DIM]
        wv_i = wv[HD * i : HD * (i + 1)]
        wq_p = wq_i.reshape(QH_PER_CORE, HD, DIM)[:, perm, :].reshape(DQ, DIM)
        wk_p = wk_i[perm, :]
        wt = np.concatenate([wq_p.T, wk_p.T, wv_i.T], axis=1).astype(bf16)
        wot = np.ascontiguousarray(wo[:, DQ * i : DQ * (i + 1)].T).astype(
            bf16
        )  # [512, DIM]
        in_maps.append(
            {
                "xt": xt,
                "wt": np.ascontiguousarray(wt),
                "wot": wot,
                "csd": csd,
                "snd": snd,
                "tri": tri,
            }
        )
    return in_maps


def _get_runner():
    global _RUNNER
    if _RUNNER is None:
        _RUNNER = _build()
    return _RUNNER


def kernel(x, cos, sin, wq, wk, wv, wo):
    from concourse.bass_utils import run_bass_kernel_spmd

    nc = _get_runner()
    in_maps = _prep_inputs(x, cos, sin, wq, wk, wv, wo)
    res = run_bass_kernel_spmd(nc, in_maps, list(range(N_CORES)))
    out = np.zeros((S, DIM), dtype=np.float32)
    for i in range(N_CORES):
        out += np.asarray(res.results[i]["out"], dtype=np.float32)
    return out[None].astype(np.float32)
